# revision 17
# baseline (speedup 1.0000x reference)
"""BidirectionalMemory kernel for 8 TRN2 NeuronCores.

Shards memory_size (M=32768) across 8 cores (4096 each). Per core:
  phase 1: dots [q,m] via fp16-split matmuls -> row-max -> AllReduce(max)
  phase 2: dots^T [m,q] rematerialized scaled by 1/t -> ln/exp ^8 chain with
           exact gating -> PV + den matmuls -> AllReduce(add) -> read_proj.
"""
import os
import sys
import numpy as np

sys.path.insert(0, "/opt/trn_rl_repo/concourse")

import concourse.bass as bass
import concourse.bacc as bacc
import concourse.mybir as mybir
import concourse.tile as tile
from concourse.bass_utils import run_bass_kernel_spmd

F32 = mybir.dt.float32
F16 = mybir.dt.float16
AF = mybir.ActivationFunctionType
ALU = mybir.AluOpType
AX = mybir.AxisListType

NCORES = 8
B, Q, QD, E, M, VD = 4, 1024, 512, 128, 32768, 512
ML = M // NCORES          # 4096 m per core
QT = (B * Q) // 128       # 32 q-tiles
MT = ML // 128            # 32 m-tiles per core

_CACHE = {}


def _build():
    nc = bacc.Bacc("TRN2", target_bir_lowering=False, debug=False,
                   num_devices=NCORES)

    qT_d = nc.dram_tensor("qT", [QD, B * Q], F32, kind="ExternalInput")
    wT_d = nc.dram_tensor("wT", [QD, E], F32, kind="ExternalInput")
    rT_d = nc.dram_tensor("rT", [E, VD], F32, kind="ExternalInput")
    keys_d = nc.dram_tensor("keys", [ML, E], F32, kind="ExternalInput")
    vals_d = nc.dram_tensor("vals", [B, ML, E], F32, kind="ExternalInput")
    id_d = nc.dram_tensor("ident", [128, 128], F32, kind="ExternalInput")
    out_d = nc.dram_tensor("out", [B * Q, VD], F32, kind="ExternalOutput")

    rg = [list(range(NCORES))]

    with tile.TileContext(nc) as tc:
        with (
            tc.tile_pool(name="big", bufs=1) as big,
            tc.tile_pool(name="work", bufs=3) as work,
            tc.tile_pool(name="small", bufs=2) as small,
            tc.tile_pool(name="ps", bufs=2, space="PSUM") as ps,
            tc.tile_pool(name="pvp", bufs=1, space="PSUM") as pvp,
            tc.tile_pool(name="dram", bufs=1, space="DRAM") as dram,
        ):
            ident = big.tile([128, 128], F32)
            nc.sync.dma_start(ident[:], id_d[:])
            rt_sb = big.tile([128, VD], F32)
            nc.sync.dma_start(rt_sb[:], rT_d[:])
            wt_sb = big.tile([128, 512], F32)
            for c in range(4):
                nc.sync.dma_start(wt_sb[:, c * 128:(c + 1) * 128],
                                  wT_d[c * 128:(c + 1) * 128, :])
            ones16 = big.tile([128, 1], F16)
            nc.vector.memset(ones16[:], 1.0)

            # persistent big tensors
            Ap = big.tile([128, B * Q], F32)      # A' [q(tile-major), e]
            AhT = big.tile([128, B * Q], F16)
            AlT = big.tile([128, B * Q], F16)
            KhT = big.tile([128, ML], F16)
            KlT = big.tile([128, ML], F16)
            GhT = big.tile([128, B * Q], F16)     # (A'/t) hi
            GlT = big.tile([128, B * Q], F16)     # (A'/t) lo
            V16 = big.tile([128, B * MT * 128], F16)
            rq_st = big.tile([128, QT], F32)
            rk_st = big.tile([128, MT], F32)
            RK4 = big.tile([100, ML], F16)
            RQ4 = big.tile([100, B * Q], F16)
            rmax = big.tile([128, QT], F32)

            KpT = big.tile([128, B * Q], F32, tag="bigT")

            def split(hi, lo, src, n):
                for z in range(0, n, 1024):
                    h32 = work.tile([128, 1024], F32, tag="h32", bufs=1)
                    zz = slice(z, z + 1024)
                    nc.vector.tensor_copy(hi[:, zz], src[:, zz])
                    nc.vector.tensor_copy(h32[:], hi[:, zz])
                    nc.vector.tensor_tensor(lo[:, zz], src[:, zz], h32[:],
                                            op=ALU.subtract)

            def rsqrt_newton(dst, x):
                # dst = 1/sqrt(x), refined
                rc = small.tile([128, 1], F32, tag="rs1")
                nc.vector.reciprocal(rc[:], x[:])
                r0 = small.tile([128, 1], F32, tag="rs2")
                nc.scalar.activation(r0[:], rc[:], AF.Sqrt)
                t1 = small.tile([128, 1], F32, tag="rs3")
                nc.vector.tensor_tensor(t1[:], r0[:], r0[:], op=ALU.mult)
                nc.vector.tensor_tensor(t1[:], t1[:], x[:], op=ALU.mult)
                nc.vector.tensor_scalar(t1[:], t1[:], -0.5, 1.5,
                                        op0=ALU.mult, op1=ALU.add)
                nc.vector.tensor_tensor(dst, r0[:], t1[:], op=ALU.mult)

            # ---- keys prep ----
            for j in range(MT):
                kst = work.tile([128, E], F32, tag="kst")
                nc.sync.dma_start(kst[:], keys_d[j * 128:(j + 1) * 128, :])
                ak = work.tile([128, E], F32, tag="ak")
                nc.scalar.activation(ak[:], kst[:], AF.Exp, scale=2.0)
                sq = work.tile([128, E], F32, tag="sq")
                nc.vector.tensor_tensor(sq[:], ak[:], ak[:], op=ALU.mult)
                ssq = small.tile([128, 1], F32, tag="ssq")
                nc.vector.tensor_reduce(ssq[:], sq[:], axis=AX.X, op=ALU.add)
                nc.vector.tensor_scalar(ssq[:], ssq[:], 1.0, None, op0=ALU.add)
                rsqrt_newton(rk_st[:, j:j + 1], ssq)
                kp = work.tile([128, E], F32, tag="kp")
                nc.vector.tensor_scalar(kp[:], ak[:], rk_st[:, j:j + 1], None,
                                        op0=ALU.mult)
                pt = ps.tile([128, 1024], F32, tag="ps")
                nc.tensor.transpose(pt[:, 0:128], kp[:], ident[:])
                nc.scalar.copy(KpT[:, j * 128:(j + 1) * 128], pt[:, 0:128])

            # split K before ApT reuses the slot
            split(KhT, KlT, KpT, ML)

            ApT = big.tile([128, B * Q], F32, tag="bigT")
            # ---- queries prep ----
            for i in range(QT):
                pj0 = ps.tile([128, 1024], F32, tag="ps")
                pj = pj0[:, 0:128]
                for c in range(4):
                    qc = work.tile([128, 128], F32, tag="qc")
                    nc.sync.dma_start(
                        qc[:], qT_d[c * 128:(c + 1) * 128,
                                    i * 128:(i + 1) * 128])
                    nc.tensor.matmul(pj, qc[:],
                                     wt_sb[:, c * 128:(c + 1) * 128],
                                     start=(c == 0), stop=(c == 3))
                aq = work.tile([128, E], F32, tag="ak")
                nc.scalar.activation(aq[:], pj, AF.Exp, scale=2.0)
                sq = work.tile([128, E], F32, tag="sq")
                nc.vector.tensor_tensor(sq[:], aq[:], aq[:], op=ALU.mult)
                ssq = small.tile([128, 1], F32, tag="ssq")
                nc.vector.tensor_reduce(ssq[:], sq[:], axis=AX.X, op=ALU.add)
                nc.vector.tensor_scalar(ssq[:], ssq[:], 1.0, None, op0=ALU.add)
                rsqrt_newton(rq_st[:, i:i + 1], ssq)
                nc.vector.tensor_scalar(Ap[:, i * 128:(i + 1) * 128], aq[:],
                                        rq_st[:, i:i + 1], None, op0=ALU.mult)
                pt = ps.tile([128, 1024], F32, tag="ps")
                nc.tensor.transpose(pt[:, 0:128], Ap[:, i * 128:(i + 1) * 128],
                                    ident[:])
                nc.scalar.copy(ApT[:, i * 128:(i + 1) * 128], pt[:, 0:128])

            # ---- V cast to fp16 ----
            for b in range(B):
                for j in range(MT):
                    vst = work.tile([128, E], F32, tag="vst")
                    nc.sync.dma_start(vst[:],
                                      vals_d[b, j * 128:(j + 1) * 128, :])
                    nc.vector.tensor_copy(
                        V16[:, (b * MT + j) * 128:(b * MT + j) * 128 + 128],
                        vst[:])

            split(AhT, AlT, ApT, B * Q)

            # rank rows: rk split to RK4 rows [rkh, rkl, rkh, rkl]
            rkh = small.tile([128, MT], F16, tag="rkh")
            rkl = small.tile([128, MT], F16, tag="rkl")
            nc.vector.tensor_copy(rkh[:], rk_st[:])
            rk32 = small.tile([128, MT], F32, tag="rk32")
            nc.vector.tensor_copy(rk32[:], rkh[:])
            nc.vector.tensor_tensor(rkl[:], rk_st[:], rk32[:], op=ALU.subtract)
            for j in range(MT):
                nc.sync.dma_start(RK4[0:1, j * 128:(j + 1) * 128],
                                  rkh[:, j:j + 1])
                nc.sync.dma_start(RK4[1:2, j * 128:(j + 1) * 128],
                                  rkl[:, j:j + 1])
            nc.sync.dma_start(RK4[2:3, :], RK4[0:1, :])
            nc.sync.dma_start(RK4[3:4, :], RK4[1:2, :])
            for g in range(1, 4):
                nc.sync.dma_start(RK4[32 * g:32 * g + 4, :], RK4[0:4, :])

            # RQ4 (phase-1, unscaled): rows [rqh, rqh, rql, rql]
            rqh = small.tile([128, QT], F16, tag="rqh")
            rql = small.tile([128, QT], F16, tag="rql")
            nc.vector.tensor_copy(rqh[:], rq_st[:])
            rq32 = small.tile([128, QT], F32, tag="rq32")
            nc.vector.tensor_copy(rq32[:], rqh[:])
            nc.vector.tensor_tensor(rql[:], rq_st[:], rq32[:], op=ALU.subtract)
            for i in range(QT):
                nc.sync.dma_start(RQ4[0:1, i * 128:(i + 1) * 128],
                                  rqh[:, i:i + 1])
                nc.sync.dma_start(RQ4[2:3, i * 128:(i + 1) * 128],
                                  rql[:, i:i + 1])
            nc.sync.dma_start(RQ4[1:2, :], RQ4[0:1, :])
            nc.sync.dma_start(RQ4[3:4, :], RQ4[2:3, :])
            for g in range(1, 4):
                nc.sync.dma_start(RQ4[32 * g:32 * g + 4, :], RQ4[0:4, :])

            # ---- phase 1: dots [q,m], row max ----
            for i in range(QT):
                hm = []
                for h in range(4):
                    p1 = ps.tile([128, 1024], F32, tag="ps")
                    for c in range(2):
                        m0 = h * 1024 + c * 512
                        o = p1[:, c * 512:(c + 1) * 512]
                        a_sl = (slice(None), slice(i * 128, (i + 1) * 128))
                        nc.tensor.matmul(o, AhT[a_sl], KhT[:, m0:m0 + 512],
                                         start=True, stop=False)
                        nc.tensor.matmul(o, AhT[a_sl], KlT[:, m0:m0 + 512],
                                         start=False, stop=False)
                        nc.tensor.matmul(o, AlT[a_sl], KhT[:, m0:m0 + 512],
                                         start=False, stop=False)
                        g = 32 * ((h * 2 + c) % 3)
                        nc.tensor.matmul(
                            o, RQ4[g:g + 4, i * 128:(i + 1) * 128],
                            RK4[g:g + 4, m0:m0 + 512],
                            start=False, stop=True)
                    rm = small.tile([128, 1], F32, tag="rm%d" % h)
                    nc.vector.tensor_reduce(rm[:], p1[:], axis=AX.X,
                                            op=ALU.max)
                    hm.append(rm)
                nc.vector.tensor_tensor(hm[0][:], hm[0][:], hm[1][:],
                                        op=ALU.max)
                nc.vector.tensor_tensor(hm[2][:], hm[2][:], hm[3][:],
                                        op=ALU.max)
                nc.vector.tensor_tensor(rmax[:, i:i + 1], hm[0][:], hm[2][:],
                                        op=ALU.max)

            # ---- AllReduce max ----
            cin = dram.tile([128, QT], F32)
            cout = dram.tile([128, QT], F32, addr_space="Shared")
            nc.sync.dma_start(cin[:], rmax[:])
            nc.gpsimd.collective_compute("AllReduce", ALU.max,
                                         replica_groups=rg,
                                         ins=[cin.opt()], outs=[cout.opt()])
            gmax = big.tile([128, QT], F32)
            nc.sync.dma_start(gmax[:], cout[:])

            # ---- thresholds ----
            m8 = small.tile([128, QT], F32, tag="m8")
            nc.vector.tensor_tensor(m8[:], gmax[:], gmax[:], op=ALU.mult)
            nc.vector.tensor_tensor(m8[:], m8[:], m8[:], op=ALU.mult)
            nc.vector.tensor_tensor(m8[:], m8[:], m8[:], op=ALU.mult)
            bb = small.tile([128, QT], F32, tag="bb")
            nc.vector.tensor_scalar(bb[:], m8[:], 0.5, None, op0=ALU.is_lt)
            thr = small.tile([128, QT], F32, tag="thr")
            nc.vector.tensor_scalar(thr[:], m8[:], 0.9, -0.5,
                                    op0=ALU.mult, op1=ALU.add)
            nc.vector.tensor_tensor(thr[:], thr[:], bb[:], op=ALU.mult)
            nc.vector.tensor_scalar(thr[:], thr[:], 0.5, None, op0=ALU.add)
            tv = small.tile([128, QT], F32, tag="tv")
            nc.scalar.activation(tv[:], thr[:], AF.Ln)
            nc.scalar.activation(tv[:], tv[:], AF.Exp, scale=0.125)
            tinv = big.tile([128, QT], F32)
            nc.vector.reciprocal(tinv[:], tv[:])

            # ---- scaled A operands for phase 2 ----
            for i in range(QT):
                ga = work.tile([128, E], F32, tag="ga")
                nc.vector.tensor_scalar(ga[:], Ap[:, i * 128:(i + 1) * 128],
                                        tinv[:, i:i + 1], None, op0=ALU.mult)
                pt = ps.tile([128, 1024], F32, tag="ps")
                nc.tensor.transpose(pt[:, 0:128], ga[:], ident[:])
                gaT = work.tile([128, 128], F32, tag="gaT")
                nc.scalar.copy(gaT[:], pt[:, 0:128])
                sl = slice(i * 128, (i + 1) * 128)
                nc.vector.tensor_copy(GhT[:, sl], gaT[:])
                g32 = work.tile([128, 128], F32, tag="g32")
                nc.vector.tensor_copy(g32[:], GhT[:, sl])
                nc.vector.tensor_tensor(GlT[:, sl], gaT[:], g32[:],
                                        op=ALU.subtract)

            # RQT4 (phase-2): rows [gh, gh, gl, gl] of rq*tinv
            rqt = small.tile([128, QT], F32, tag="rqt")
            nc.vector.tensor_tensor(rqt[:], rq_st[:], tinv[:], op=ALU.mult)
            qh = small.tile([128, QT], F16, tag="qh")
            ql = small.tile([128, QT], F16, tag="ql")
            nc.vector.tensor_copy(qh[:], rqt[:])
            q32 = small.tile([128, QT], F32, tag="q32")
            nc.vector.tensor_copy(q32[:], qh[:])
            nc.vector.tensor_tensor(ql[:], rqt[:], q32[:], op=ALU.subtract)
            RQT4 = big.tile([100, B * Q], F16)
            for i in range(QT):
                nc.sync.dma_start(RQT4[0:1, i * 128:(i + 1) * 128],
                                  qh[:, i:i + 1])
                nc.sync.dma_start(RQT4[2:3, i * 128:(i + 1) * 128],
                                  ql[:, i:i + 1])
            nc.sync.dma_start(RQT4[1:2, :], RQT4[0:1, :])
            nc.sync.dma_start(RQT4[3:4, :], RQT4[2:3, :])
            for g in range(1, 4):
                nc.sync.dma_start(RQT4[32 * g:32 * g + 4, :], RQT4[0:4, :])

            # ---- phase 2 + PV per batch-quarter ----
            pv_sb = big.tile([128, B * Q], F32, tag="pvs")
            den_sb = big.tile([128, QT], F32)
            for b in range(B):
                q0 = b * Q
                pv = pvp.tile([128, 1024], F32, tag="pv")
                dn = pvp.tile([1, 1024], F32, tag="dn")
                for j in range(MT):
                    p2 = ps.tile([128, 1024], F32, tag="ps")
                    ksl = (slice(None), slice(j * 128, (j + 1) * 128))
                    for c in range(2):
                        qs0 = q0 + c * 512
                        o = p2[:, c * 512:(c + 1) * 512]
                        nc.tensor.matmul(o, KhT[ksl], GhT[:, qs0:qs0 + 512],
                                         start=True, stop=False)
                        nc.tensor.matmul(o, KhT[ksl], GlT[:, qs0:qs0 + 512],
                                         start=False, stop=False)
                        nc.tensor.matmul(o, KlT[ksl], GhT[:, qs0:qs0 + 512],
                                         start=False, stop=False)
                        g = 32 * ((j * 2 + c) % 3)
                        nc.tensor.matmul(
                            o, RK4[g:g + 4, j * 128:(j + 1) * 128],
                            RQT4[g:g + 4, qs0:qs0 + 512],
                            start=False, stop=True)
                    l16 = work.tile([128, 1024], F16, tag="l16", bufs=2)
                    nc.scalar.activation(l16[:], p2[:], AF.Ln)
                    e16 = work.tile([128, 1024], F16, tag="e16", bufs=2)
                    nc.scalar.activation(e16[:], l16[:], AF.Exp, scale=8.0)
                    m16 = work.tile([128, 1024], F16, tag="m16", bufs=2)
                    nc.vector.tensor_scalar(m16[:], l16[:], 0.0, None,
                                            op0=ALU.is_ge)
                    gp = work.tile([128, 1024], F16, tag="gp", bufs=2)
                    nc.vector.tensor_scalar(gp[:], e16[:], -1.0, 0.0,
                                            op0=ALU.add, op1=ALU.max)
                    gg = work.tile([128, 1024], F16, tag="gg", bufs=2)
                    nc.vector.tensor_tensor(gg[:], gp[:], m16[:], op=ALU.add)
                    vsl = (slice(None),
                           slice((b * MT + j) * 128, (b * MT + j) * 128 + 128))
                    for c in range(2):
                        nc.tensor.matmul(pv[:, c * 512:(c + 1) * 512],
                                         V16[vsl], gg[:, c * 512:(c + 1) * 512],
                                         start=(j == 0), stop=(j == MT - 1))
                        nc.tensor.matmul(dn[0:1, c * 512:(c + 1) * 512],
                                         ones16[:], gg[:, c * 512:(c + 1) * 512],
                                         start=(j == 0), stop=(j == MT - 1))
                nc.vector.tensor_copy(pv_sb[:, q0:q0 + 1024], pv[:])
                dtmp = work.tile([1, 1024], F32, tag="dtmp", bufs=1)
                nc.scalar.copy(dtmp[:], dn[:])
                for t in range(8):
                    nc.sync.dma_start(
                        den_sb[:, b * 8 + t:b * 8 + t + 1],
                        dtmp[0:1, t * 128:(t + 1) * 128])

            # ---- AllReduce add of [pv ; den] ----
            sin = dram.tile([129, B * Q], F32)
            sout = dram.tile([129, B * Q], F32, addr_space="Shared")
            nc.sync.dma_start(sin[0:128, :], pv_sb[:])
            for i in range(QT):
                nc.sync.dma_start(sin[128:129, i * 128:(i + 1) * 128],
                                  den_sb[:, i:i + 1])
            nc.gpsimd.collective_compute("AllReduce", ALU.add,
                                         replica_groups=rg,
                                         ins=[sin.opt()], outs=[sout.opt()])
            pvg = big.tile([128, B * Q], F32, tag="pvs")
            deng = big.tile([128, QT], F32)
            nc.sync.dma_start(pvg[:], sout[0:128, :])
            for i in range(QT):
                nc.sync.dma_start(deng[:, i:i + 1],
                                  sout[128:129, i * 128:(i + 1) * 128])

            # ---- final read_proj + divide ----
            for i in range(QT):
                pf0 = ps.tile([128, 1024], F32, tag="ps")
                pf = pf0[:, 0:VD]
                nc.tensor.matmul(pf, pvg[:, i * 128:(i + 1) * 128],
                                 rt_sb[:], start=True, stop=True)
                rc = small.tile([128, 1], F32, tag="rc")
                nc.vector.reciprocal(rc[:], deng[:, i:i + 1])
                ot = work.tile([128, VD], F32, tag="ot", bufs=2)
                nc.vector.tensor_scalar(ot[:], pf, rc[:], None,
                                        op0=ALU.mult)
                nc.sync.dma_start(out_d[i * 128:(i + 1) * 128, :], ot[:])

    nc.compile()
    return nc


def kernel(**inputs):
    queries = inputs["queries"].astype(np.float32)
    W = inputs["query_proj_w"].astype(np.float32)
    keys = inputs["memory_keys_raw"].astype(np.float32)
    vals = inputs["memory_values"].astype(np.float32)
    R = inputs["read_proj_w"].astype(np.float32)

    if "nc" not in _CACHE:
        _CACHE["nc"] = _build()
    nc = _CACHE["nc"]

    qT = np.ascontiguousarray(queries.reshape(B * Q, QD).T)
    wT = np.ascontiguousarray(W.T)
    rT = np.ascontiguousarray(R.T)
    ident = np.eye(128, dtype=np.float32)

    in_maps = []
    for c in range(NCORES):
        sl = slice(c * ML, (c + 1) * ML)
        in_maps.append(dict(
            qT=qT, wT=wT, rT=rT, ident=ident,
            keys=np.ascontiguousarray(keys[sl]),
            vals=np.ascontiguousarray(vals[:, sl]),
        ))

    trace = bool(int(os.environ.get("TRACE_KERNEL", "0")))
    res = run_bass_kernel_spmd(nc, in_maps, core_ids=list(range(NCORES)),
                               trace=trace)
    if trace:
        _CACHE["exec_time_ns"] = res.exec_time_ns
        _CACHE["trace"] = res.instructions_and_trace
    return res.results[0]["out"].reshape(B, Q, VD)
